# revision 2
# baseline (speedup 1.0000x reference)
# Trainium2 Bass kernel for nn_Decoder (LSTM decoder + GCN message passing).
#
# Strategy (8 NeuronCores, SPMD):
#   * Data-parallel over nodes N=10000 -> 1250 nodes/core for fc2 + LSTM +
#     projection. State kept feature-major ([H, nodes]) so every matmul is
#     PE-friendly with K=H=128 and no transposes.
#   * Algebraic rewrite: the GCN aggregation and fc3 are both linear, so
#     aggregate AFTER projecting features to NF=16:
#        x_hat[n,t] = dinv[n] * sum_{e: dst=n} (dinv[src] * mask[src] * hs[t,src] @ (W_gcn@W_fc3))
#                     + (b_gcn@W_fc3 + b_fc3)
#     This shrinks the scatter/gather payload 8x (H=128 -> NF=16 per t).
#   * Y table ([N, T*NF], fp16, dinv*mask pre-scaled) is AllGather'ed across
#     the 8 cores; each core then aggregates edges partitioned by destination
#     tile (128 dst nodes per tile) using indirect-DMA row gathers and a
#     one-hot selection matmul that accumulates in PSUM (vertex-cut scheme).
import os
import numpy as np

import concourse.bass as bass
import concourse.bacc as bacc
import concourse.tile as tile
from concourse import mybir
from concourse import bass_utils

P = 128
N, T, NF, H, L, E = 10000, 12, 16, 128, 64, 160000
NCORES = 8
NCN = N // NCORES            # 1250 nodes per core
NTILES = (NCN + P - 1) // P  # 10 dst tiles per core
CH = [(0, 512), (512, 512), (1024, NCN - 1024)]  # LSTM node chunks (<=512)
TNF = T * NF                 # 192

F32 = mybir.dt.float32
F16 = mybir.dt.float16
U8 = mybir.dt.uint8
I32 = mybir.dt.int32

# gate q: 0=i, 1=f, 2=g, 3=o ; activation: sigmoid for i,f,o ; tanh for g
GATE_FUNCS = ["Sigmoid", "Sigmoid", "Tanh", "Sigmoid"]

_BUILD_CACHE = {}
LAST_RESULTS = None  # BassKernelResults of the most recent run (for test harness)


def _build(b_max: int):
    nblk = NTILES * b_max
    nc = bacc.Bacc("TRN2", target_bir_lowering=False, debug=False,
                   num_devices=NCORES)

    # ---------------- I/O declarations ----------------
    zT = nc.dram_tensor("zT", [L, NCN], F32, kind="ExternalInput")
    xm = nc.dram_tensor("xm", [NCN, TNF], U8, kind="ExternalInput")
    wfc2 = nc.dram_tensor("wfc2", [L, H], F32, kind="ExternalInput")
    b2 = nc.dram_tensor("b2", [P, 1], F32, kind="ExternalInput")
    wih = nc.dram_tensor("wih", [H, 4 * H], F32, kind="ExternalInput")
    whh = nc.dram_tensor("whh", [H, 4 * H], F32, kind="ExternalInput")
    bg = nc.dram_tensor("bg", [P, 4], F32, kind="ExternalInput")
    wcomb = nc.dram_tensor("wcomb", [H, NF], F32, kind="ExternalInput")
    bout = nc.dram_tensor("bout", [P, TNF], F32, kind="ExternalInput")
    dinvt = nc.dram_tensor("dinvt", [P, NTILES], F32, kind="ExternalInput")
    iota = nc.dram_tensor("iota", [P, P], F16, kind="ExternalInput")
    esrc = nc.dram_tensor("esrc", [P, nblk], I32, kind="ExternalInput")
    erel = nc.dram_tensor("erel", [P, nblk], F32, kind="ExternalInput")
    xhat = nc.dram_tensor("xhat", [NCN, TNF], F32, kind="ExternalOutput")

    with tile.TileContext(nc) as tc:
        with tc.tile_pool(name="cpool", bufs=1) as cp, \
             tc.tile_pool(name="spool", bufs=1) as sp, \
             tc.tile_pool(name="dram", bufs=1, space="DRAM") as dp:

            # ---- constant loads ----
            zt_sb = cp.tile([L, NCN], F32)
            nc.sync.dma_start(zt_sb[:], zT[:])
            wfc2_sb = cp.tile([L, H], F32)
            nc.sync.dma_start(wfc2_sb[:], wfc2[:])
            b2_sb = cp.tile([P, 1], F32)
            nc.sync.dma_start(b2_sb[:], b2[:])
            wih_sb = cp.tile([H, 4 * H], F32)
            nc.sync.dma_start(wih_sb[:], wih[:])
            whh_sb = cp.tile([H, 4 * H], F32)
            nc.sync.dma_start(whh_sb[:], whh[:])
            bg_sb = cp.tile([P, 4], F32)
            nc.sync.dma_start(bg_sb[:], bg[:])
            wcomb_sb = cp.tile([H, NF], F32)
            nc.sync.dma_start(wcomb_sb[:], wcomb[:])
            bout_sb = cp.tile([P, TNF], F32)
            nc.sync.dma_start(bout_sb[:], bout[:])
            dinv_sb = cp.tile([P, NTILES], F32)
            nc.sync.dma_start(dinv_sb[:], dinvt[:])
            iota_sb = cp.tile([P, P], F16)
            nc.sync.dma_start(iota_sb[:], iota[:])
            esrc_sb = cp.tile([P, nblk], I32)
            nc.sync.dma_start(esrc_sb[:], esrc[:])
            erel_sb = cp.tile([P, nblk], F32)
            nc.sync.dma_start(erel_sb[:], erel[:])

            yshard = dp.tile([NCN, TNF], F16)
            yfull = dp.tile([N, TNF], F16, addr_space="Shared")

            # ---- node mask * dinv (per node-block) ----
            mdv_sb = sp.tile([P, NTILES], F32)
            with tc.tile_pool(name="wp0", bufs=3) as wp0:
                for k in range(NTILES):
                    rows = min(P, NCN - k * P)
                    xmu = wp0.tile([P, TNF], U8, tag="xmu", bufs=3)
                    nc.sync.dma_start(xmu[:rows], xm[k * P:k * P + rows, :])
                    xmf = wp0.tile([P, TNF], F32, tag="xmf", bufs=3)
                    nc.vector.tensor_copy(out=xmf[:rows], in_=xmu[:rows])
                    mx = wp0.tile([P, 1], F32, tag="mx", bufs=3)
                    nc.vector.reduce_max(out=mx[:rows], in_=xmf[:rows],
                                         axis=mybir.AxisListType.X)
                    # mask * dinv for this block
                    nc.vector.tensor_mul(out=mdv_sb[:rows, k:k + 1],
                                         in0=mx[:rows],
                                         in1=dinv_sb[:rows, k:k + 1])

            # ---- hd = z @ W_fc2 + b_fc2 (feature-major: hdT [H, nodes]) ----
            hdT = sp.tile([H, NCN], F32)
            with tc.tile_pool(name="psI", bufs=2, space="PSUM") as psI:
                for off, sz in CH:
                    ph = psI.tile([P, 512], F32, tag="ph", bufs=2)
                    nc.tensor.matmul(out=ph[:, :sz], lhsT=wfc2_sb[:],
                                     rhs=zt_sb[:, off:off + sz],
                                     start=True, stop=True)
                    nc.scalar.activation(
                        out=hdT[:, off:off + sz], in_=ph[:, :sz],
                        func=mybir.ActivationFunctionType.Identity,
                        bias=b2_sb[:, :1])

            # ---- LSTM (T steps, feature-major state) ----
            ct = []
            for j, (off, sz) in enumerate(CH):
                c_j = sp.tile([P, sz], F32, name=f"c_{j}", tag=f"c_{j}")
                nc.vector.memset(c_j[:], 0.0)
                ct.append(c_j)

            hs = []  # hs[t][j] tiles
            with tc.tile_pool(name="psG", bufs=2, space="PSUM") as psG, \
                 tc.tile_pool(name="wpL", bufs=3) as wpL:
                for t in range(T):
                    hrow = []
                    for j, (off, sz) in enumerate(CH):
                        prev = hdT[:, off:off + sz] if t == 0 else hs[t - 1][j][:]
                        pg = psG.tile([P, 4 * 512], F32, tag="pg", bufs=2)
                        for q in range(4):
                            wsl = slice(q * H, (q + 1) * H)
                            nc.tensor.matmul(out=pg[:, q * 512:q * 512 + sz],
                                             lhsT=wih_sb[:, wsl],
                                             rhs=hdT[:, off:off + sz],
                                             start=True, stop=False)
                            nc.tensor.matmul(out=pg[:, q * 512:q * 512 + sz],
                                             lhsT=whh_sb[:, wsl],
                                             rhs=prev,
                                             start=False, stop=True)
                        sg = []
                        for q in range(4):
                            s_q = wpL.tile([P, sz], F32, tag=f"sg{q}", bufs=2)
                            nc.scalar.activation(
                                out=s_q[:], in_=pg[:, q * 512:q * 512 + sz],
                                func=getattr(mybir.ActivationFunctionType,
                                             GATE_FUNCS[q]),
                                bias=bg_sb[:, q:q + 1])
                            sg.append(s_q)
                        tmp = wpL.tile([P, sz], F32, tag="tmp", bufs=2)
                        nc.vector.tensor_mul(out=tmp[:], in0=sg[0], in1=sg[2])
                        nc.vector.tensor_mul(out=ct[j][:], in0=ct[j][:], in1=sg[1])
                        nc.vector.tensor_add(out=ct[j][:], in0=ct[j][:], in1=tmp[:])
                        thc = wpL.tile([P, sz], F32, tag="thc", bufs=2)
                        nc.scalar.activation(
                            out=thc[:], in_=ct[j][:],
                            func=mybir.ActivationFunctionType.Tanh)
                        h_j = sp.tile([P, sz], F32, name=f"h_{t}_{j}",
                                      tag=f"h_{t}_{j}")
                        nc.vector.tensor_mul(out=h_j[:], in0=sg[3], in1=thc[:])
                        hrow.append(h_j)
                    hs.append(hrow)

            # ---- projection: Y[n, t*16:(t+1)*16] = mdv[n] * (hs[t,n] @ Wcomb) ----
            with tc.tile_pool(name="psY", bufs=4, space="PSUM") as psY, \
                 tc.tile_pool(name="wpY", bufs=2) as wpY:
                for k in range(NTILES):
                    rows = min(P, NCN - k * P)
                    # locate chunk j and local offset for this node block
                    goff = k * P
                    j = 0 if goff < 512 else (1 if goff < 1024 else 2)
                    loff = goff - CH[j][0]
                    py = psY.tile([P, TNF], F32, tag="py", bufs=4)
                    for t in range(T):
                        nc.tensor.matmul(out=py[:rows, t * NF:(t + 1) * NF],
                                         lhsT=hs[t][j][:, loff:loff + rows],
                                         rhs=wcomb_sb[:],
                                         start=True, stop=True)
                    ysb = wpY.tile([P, TNF], F16, tag="ysb", bufs=2)
                    nc.vector.tensor_scalar(out=ysb[:rows], in0=py[:rows],
                                            scalar1=mdv_sb[:rows, k:k + 1],
                                            scalar2=None,
                                            op0=mybir.AluOpType.mult)
                    nc.sync.dma_start(yshard[k * P:k * P + rows, :], ysb[:rows])

            # ---- AllGather the projected features ----
            nc.gpsimd.collective_compute(
                "AllGather", mybir.AluOpType.bypass,
                replica_groups=[list(range(NCORES))],
                ins=[yshard.opt()], outs=[yfull.opt()],
            )

            # ---- GCN aggregation per destination tile ----
            with tc.tile_pool(name="psC", bufs=2, space="PSUM") as psC, \
                 tc.tile_pool(name="wpC", bufs=6) as wpC:
                for k in range(NTILES):
                    rows = min(P, NCN - k * P)
                    pa = psC.tile([P, TNF], F32, tag="pa", bufs=2)
                    for b in range(b_max):
                        col = k * b_max + b
                        yg = wpC.tile([P, TNF], F16, tag="yg", bufs=6)
                        nc.gpsimd.indirect_dma_start(
                            out=yg[:], out_offset=None,
                            in_=yfull[:, :],
                            in_offset=bass.IndirectOffsetOnAxis(
                                ap=esrc_sb[:, col:col + 1], axis=0))
                        sel = wpC.tile([P, P], F16, tag="sel", bufs=6)
                        nc.vector.tensor_scalar(out=sel[:], in0=iota_sb[:],
                                                scalar1=erel_sb[:, col:col + 1],
                                                scalar2=None,
                                                op0=mybir.AluOpType.is_equal)
                        nc.tensor.matmul(out=pa[:], lhsT=sel[:], rhs=yg[:],
                                         start=(b == 0), stop=(b == b_max - 1))
                    osb = wpC.tile([P, TNF], F32, tag="osb", bufs=2)
                    nc.vector.tensor_scalar(out=osb[:rows], in0=pa[:rows],
                                            scalar1=dinv_sb[:rows, k:k + 1],
                                            scalar2=None,
                                            op0=mybir.AluOpType.mult)
                    nc.vector.tensor_add(out=osb[:rows], in0=osb[:rows],
                                         in1=bout_sb[:rows])
                    nc.sync.dma_start(xhat[k * P:k * P + rows, :], osb[:rows])

    nc.compile()
    return nc


def _preprocess(z, edge_index, x_mask, W_fc2, b_fc2, W_ih, W_hh, b_ih, b_hh,
                W_gcn, b_gcn, W_fc3, b_fc3):
    z = np.asarray(z, np.float32)
    edge_index = np.asarray(edge_index).astype(np.int64)
    x_mask = np.asarray(x_mask)
    src = edge_index[0]
    dst = edge_index[1]
    deg = (np.bincount(dst, minlength=N) + 1.0)
    dinv = (1.0 / np.sqrt(deg)).astype(np.float32)

    src_all = np.concatenate([src, np.arange(N, dtype=np.int64)])
    dst_all = np.concatenate([dst, np.arange(N, dtype=np.int64)])

    core_of = dst_all // NCN
    rel_in_core = dst_all % NCN
    tile_of = rel_in_core // P
    relp = rel_in_core % P

    key = core_of * NTILES + tile_of
    order = np.argsort(key, kind="stable")
    key_s = key[order]
    cnts = np.bincount(key_s, minlength=NCORES * NTILES)
    b_max = int(np.max((cnts + P - 1) // P))
    nblk = NTILES * b_max

    e_src = np.zeros((NCORES, P, nblk), np.int32)
    e_rel = np.full((NCORES, P, nblk), 200.0, np.float32)
    starts = np.concatenate([[0], np.cumsum(cnts)])
    for c in range(NCORES):
        for k in range(NTILES):
            kk = c * NTILES + k
            idx = order[starts[kk]:starts[kk + 1]]
            n = len(idx)
            bcol = np.arange(n) // P
            prow = np.arange(n) % P
            e_src[c, prow, k * b_max + bcol] = src_all[idx]
            e_rel[c, prow, k * b_max + bcol] = relp[idx].astype(np.float32)

    Wcomb = np.ascontiguousarray((np.asarray(W_gcn, np.float32)
                                  @ np.asarray(W_fc3, np.float32)))
    bias16 = (np.asarray(b_gcn, np.float32) @ np.asarray(W_fc3, np.float32)
              + np.asarray(b_fc3, np.float32))
    bout_t = np.ascontiguousarray(np.tile(bias16, (P, T)).astype(np.float32))
    bgv = (np.asarray(b_ih, np.float32) + np.asarray(b_hh, np.float32))
    bg_t = np.ascontiguousarray(bgv.reshape(4, P).T.astype(np.float32))
    b2_t = np.ascontiguousarray(np.asarray(b_fc2, np.float32)
                                .reshape(P, 1))
    wih_t = np.ascontiguousarray(np.asarray(W_ih, np.float32).T)
    whh_t = np.ascontiguousarray(np.asarray(W_hh, np.float32).T)
    wfc2_t = np.ascontiguousarray(np.asarray(W_fc2, np.float32))
    iota_t = np.ascontiguousarray(
        np.broadcast_to(np.arange(P, dtype=np.float16)[None, :], (P, P)))

    dinv_t = np.zeros((NCORES, P, NTILES), np.float32)
    xm_t = np.zeros((NCORES, NCN, TNF), np.uint8)
    zT_t = np.zeros((NCORES, L, NCN), np.float32)
    for c in range(NCORES):
        sl = slice(c * NCN, (c + 1) * NCN)
        dv = dinv[sl]
        for k in range(NTILES):
            rows = min(P, NCN - k * P)
            dinv_t[c, :rows, k] = dv[k * P:k * P + rows]
        xm_t[c] = x_mask[sl].reshape(NCN, TNF).astype(np.uint8)
        zT_t[c] = z[sl].T

    in_maps = []
    for c in range(NCORES):
        in_maps.append({
            "zT": np.ascontiguousarray(zT_t[c]),
            "xm": np.ascontiguousarray(xm_t[c]),
            "wfc2": wfc2_t,
            "b2": b2_t,
            "wih": wih_t,
            "whh": whh_t,
            "bg": bg_t,
            "wcomb": Wcomb,
            "bout": bout_t,
            "dinvt": np.ascontiguousarray(dinv_t[c]),
            "iota": iota_t,
            "esrc": np.ascontiguousarray(e_src[c]),
            "erel": np.ascontiguousarray(e_rel[c]),
        })
    return b_max, in_maps


def kernel(z, edge_index, x_mask, W_fc2, b_fc2, W_ih, W_hh, b_ih, b_hh,
           W_gcn, b_gcn, W_fc3, b_fc3):
    global LAST_RESULTS
    b_max, in_maps = _preprocess(z, edge_index, x_mask, W_fc2, b_fc2,
                                 W_ih, W_hh, b_ih, b_hh,
                                 W_gcn, b_gcn, W_fc3, b_fc3)
    if b_max not in _BUILD_CACHE:
        _BUILD_CACHE[b_max] = _build(b_max)
    nc = _BUILD_CACHE[b_max]

    trace = bool(int(os.environ.get("KERNEL_TRACE", "0")))
    res = bass_utils.run_bass_kernel_spmd(
        nc, in_maps, core_ids=list(range(NCORES)), trace=trace)
    LAST_RESULTS = res

    out = np.empty((N, T, NF), np.float32)
    for c in range(NCORES):
        out[c * NCN:(c + 1) * NCN] = res.results[c]["xhat"].reshape(NCN, T, NF)
    return out
